# revision 18
# baseline (speedup 1.0000x reference)
"""Trainium2 Bass kernel for nn_Decoder: 16-step GRU decoder, vocab-parallel
across 8 NeuronCores.

v2 layout: everything batch-on-partitions where it kills LDWEIGHTS churn.
  - logits in (B, vocab): lhsT = h1T k-tiles (8 stationary loads/step), rhs =
    woutT (H-part, vocab-free) streamed N=512. 64 matmuls vs 256 weight
    reloads of the (vocab, B) form.
  - softmax over batch: exp computed on (B,v) chunks, DMA-transposed
    (XBAR) to (v,B) tiles off the PE; batch sums = free-axis vector reduce.
  - xpart probs @ w_in in fp8 e4m3 DoubleRow (2 k-tiles/instr). probs*64,
    w_in*2^14 to stay in e4m3 normal range; descaled 2^-20 on drain.
  - GRU gates in (B, 3H/8) form: lhsT = xT/hT k-tiles, rhs = W chunks.
    Both layers' gate psums live in one bank each: [rz 256 | gi_n 128 |
    gh_n 128]; biases preloaded via a K=1 ones-row matmul so accumulation
    lands on them. ghh halves run inside the AllReduce window.
  - output DMAed as bf16 (B, vocab) chunks, host converts to f32.
Collectives per step: AllGather h0, AllGather h1 (32KB in), AllReduce x
(256KB bf16). Heater matmuls keep the PE HAM clock up inside collective
windows.
"""
import numpy as np
import ml_dtypes

import concourse.bass as bass
import concourse.mybir as mybir
import concourse.tile as tile
from concourse import bacc
from concourse import bass_utils

F32 = mybir.dt.float32
BF16 = mybir.dt.bfloat16
FP8 = mybir.dt.float8e4
AF = mybir.ActivationFunctionType
ALU = mybir.AluOpType
DR = mybir.MatmulPerfMode.DoubleRow

B = 128
H = 1024
V = 32000
T = 16
BOS = 1
NC = 8
VS = 4096          # per-core padded vocab
VT = VS // 128     # 32 vocab tiles per core
NCH = VS // 512    # 8 logits chunks per core
KH = H // 128      # 8 H k-tiles
VPAD = NC * VS     # 32768
PSC = 2.0 ** 6     # probs fp8 scale
WSC = 2.0 ** 14    # w_in fp8 scale
XDESC = 1.0 / (PSC * WSC)


def build_nc(steps=T, n_cores=NC, heat_ar=48, heat_ag0=44, heat_ag1=44,
             enable_asserts=False):
    nc = bacc.Bacc("TRN2", target_bir_lowering=False, debug=False,
                   num_devices=n_cores, enable_asserts=enable_asserts)
    rg = [list(range(n_cores))]

    # ---- DRAM I/O ----
    d_woutT = nc.dram_tensor("woutT", [128, KH, VS], BF16, kind="ExternalInput").ap()
    d_boutb = nc.dram_tensor("boutb", [128, VS], BF16, kind="ExternalInput").ap()
    d_win8 = nc.dram_tensor("win8", [128, VT, H], FP8, kind="ExternalInput").ap()
    d_g = {}
    for nm in ("gih0", "ghh0", "gih1", "ghh1"):
        d_g[nm] = nc.dram_tensor(nm, [128, KH, 384], BF16, kind="ExternalInput").ap()
    d_brow0 = nc.dram_tensor("brow0", [1, 512], BF16, kind="ExternalInput").ap()
    d_brow1 = nc.dram_tensor("brow1", [1, 512], BF16, kind="ExternalInput").ap()
    d_ones = nc.dram_tensor("ones", [1, 128], BF16, kind="ExternalInput").ap()
    d_c64 = nc.dram_tensor("c64", [128, 128], BF16, kind="ExternalInput").ap()
    d_x0T = nc.dram_tensor("x0T", [128, KH, B], BF16, kind="ExternalInput").ap()
    d_h0f = nc.dram_tensor("h0f", [128, KH, B], F32, kind="ExternalInput").ap()
    d_h1f = nc.dram_tensor("h1f", [128, KH, B], F32, kind="ExternalInput").ap()
    d_h0own = nc.dram_tensor("h0own", [128, 128], F32, kind="ExternalInput").ap()
    d_h1own = nc.dram_tensor("h1own", [128, 128], F32, kind="ExternalInput").ap()
    d_ident = nc.dram_tensor("ident", [128, 128], BF16, kind="ExternalInput").ap()
    d_out = nc.dram_tensor("logits", [steps, 128, VS], BF16, kind="ExternalOutput").ap()

    with tile.TileContext(nc) as tc:
        with tc.tile_pool(name="wpool", bufs=1) as wpool, \
             tc.tile_pool(name="state", bufs=1) as state, \
             tc.tile_pool(name="sb", bufs=3) as sb, \
             tc.tile_pool(name="ps", bufs=1, space="PSUM") as ps, \
             tc.tile_pool(name="dram", bufs=2, space="DRAM") as dram:

            # ---- weights to SBUF; GRU weights first so step 0 starts early
            g_sb = {}
            for nm in ("gih0", "ghh0", "gih1", "ghh1"):
                t_ = wpool.tile([128, KH, 384], BF16, name=nm + "_sb")
                nc.sync.dma_start(t_[:], d_g[nm][:])
                g_sb[nm] = t_
            brow = []
            for l, d_b in enumerate((d_brow0, d_brow1)):
                t_ = wpool.tile([1, 512], BF16, name=f"brow{l}_sb")
                nc.sync.dma_start(t_[:], d_b[:])
                brow.append(t_)
            ones = wpool.tile([1, 128], BF16)
            nc.sync.dma_start(ones[:], d_ones[:])
            c64 = wpool.tile([128, 128], BF16)
            nc.sync.dma_start(c64[:], d_c64[:])
            ident = wpool.tile([128, 128], BF16)
            nc.sync.dma_start(ident[:], d_ident[:])
            boutb = wpool.tile([128, VS], BF16)
            nc.sync.dma_start(boutb[:], d_boutb[:])
            x0_sb = state.tile([128, KH, B], BF16)
            nc.sync.dma_start(x0_sb[:], d_x0T[:])

            # collective warmup
            warm_sb = sb.tile([128, 2], BF16, tag="ccwarm", bufs=1)
            nc.vector.tensor_copy(out=warm_sb[:], in_=ident[:, 0:2])
            warm_in = dram.tile([128, 2], BF16, tag="ccwin", bufs=1)
            warm_ago = dram.tile([n_cores * 128, 2], BF16, tag="ccwago", bufs=1)
            warm_aro = dram.tile([128, 2], BF16, tag="ccwaro", bufs=1)
            nc.sync.dma_start(warm_in[:], warm_sb[:])
            nc.gpsimd.collective_compute(
                "AllGather", ALU.bypass, replica_groups=rg,
                ins=[warm_in.opt()], outs=[warm_ago.opt()])
            nc.gpsimd.collective_compute(
                "AllReduce", ALU.add, replica_groups=rg,
                ins=[warm_in.opt()], outs=[warm_aro.opt()])

            wout_sb = wpool.tile([128, KH, VS], BF16)
            for k in range(KH):
                nc.sync.dma_start(wout_sb[:, k, :], d_woutT[:, k, :])
            win8 = wpool.tile([128, VT, H], FP8)
            for vo in range(0, VT, 8):
                nc.sync.dma_start(win8[:, vo:vo + 8, :], d_win8[:, vo:vo + 8, :])

            # ---- state ----
            hf = []    # full hidden (H,B) bf16 [128, KH, B]
            hown = []  # own chunk, (B, own-cols) fp32 [128, 128]
            for l, (dfull, downn) in enumerate(((d_h0f, d_h0own), (d_h1f, d_h1own))):
                tmp = sb.tile([128, KH, B], F32, tag="hstage", bufs=1, name=f"hinit{l}")
                nc.sync.dma_start(tmp[:], dfull[:])
                fb = state.tile([128, KH, B], BF16, name=f"h{l}fb")
                nc.vector.tensor_copy(out=fb[:], in_=tmp[:])
                hf.append(fb)
                own = state.tile([128, 128], F32, name=f"h{l}own")
                nc.sync.dma_start(own[:], downn[:])
                hown.append(own)

            xgb = state.tile([128, H], BF16)      # AR result, (B,H)
            xbf = state.tile([128, KH, B], BF16)  # x transposed, (H,B)
            expT8 = state.tile([128, VT, B], FP8)   # probs*64, (v,B)
            sums = state.tile([128, VT], F32)
            recs = state.tile([128, VT], F32)

            gps = [None, None]   # gate psum per layer [128,512]: rz|gi_n|gh_n

            def heat(n, t, where, anchor):
                # junk matmuls keeping the PE HAM window busy in collective
                # gaps; anchored via lhsT to a tile written at gap start
                for i in range(n):
                    hps = ps.tile([128, 512], F32, tag="lg", bufs=2,
                                  name=f"heat_{where}_{t}_{i}")
                    nc.tensor.matmul(hps[:], anchor, x0_sb[:, 0:4, :],
                                     start=True, stop=True)

            def emit_gate_prologue(l, t):
                """ones-row bias preload + ghh half; runs in the AR window
                (hf[l] holds h_l of the previous step)."""
                g = ps.tile([128, 512], F32, tag=f"gates{l}", name=f"g{l}_{t}")
                nc.tensor.matmul(g[:, 0:512], ones[:], brow[l][:],
                                 start=True, stop=False)
                ghh = g_sb[f"ghh{l}"]
                for k in range(KH):
                    nc.tensor.matmul(g[:, 0:256], hf[l][:, k, :],
                                     ghh[:, k, 0:256], start=False, stop=False)
                    nc.tensor.matmul(g[:, 384:512], hf[l][:, k, :],
                                     ghh[:, k, 256:384], start=False,
                                     stop=(k == KH - 1))
                gps[l] = g

            def emit_gih(l, t, lhsT_tiles):
                g = gps[l]
                gih = g_sb[f"gih{l}"]
                for k in range(KH):
                    nc.tensor.matmul(g[:, 0:256], lhsT_tiles[k],
                                     gih[:, k, 0:256], start=False,
                                     stop=(k == KH - 1))
                    nc.tensor.matmul(g[:, 256:384], lhsT_tiles[k],
                                     gih[:, k, 256:384], start=False,
                                     stop=(k == KH - 1))

            def emit_ew_ag(l, t):
                """gates -> new h_l; own slice transposed + AllGathered.
                h' = z*h + (1-z)*n with (1-z) = sigmoid(-gate_z), so the
                post-tanh chain is 2 ops and z*h runs off-chain."""
                g = gps[l]
                rz = sb.tile([128, 256], F32, tag="ew_rz", bufs=2, name=f"rz{l}_{t}")
                nc.scalar.activation(rz[:], g[:, 0:256], AF.Sigmoid)
                zneg = sb.tile([128, 128], F32, tag="ew_zn", bufs=2, name=f"zng{l}_{t}")
                nc.scalar.activation(zneg[:], g[:, 128:256], AF.Sigmoid, scale=-1.0)
                rhn = sb.tile([128, 128], F32, tag="ew_rhn", bufs=2, name=f"rhn{l}_{t}")
                nc.vector.tensor_mul(out=rhn[:], in0=g[:, 384:512], in1=rz[:, 0:128])
                pre = sb.tile([128, 128], F32, tag="ew_pre", bufs=2, name=f"pre{l}_{t}")
                nc.vector.tensor_add(out=pre[:], in0=rhn[:], in1=g[:, 256:384])
                t1 = sb.tile([128, 128], F32, tag="ew_t1", bufs=2, name=f"t1{l}_{t}")
                nc.vector.tensor_mul(out=t1[:], in0=rz[:, 128:256], in1=hown[l][:])
                n = sb.tile([128, 128], F32, tag="ew_n", bufs=2, name=f"n{l}_{t}")
                nc.scalar.activation(n[:], pre[:], AF.Tanh)
                zs = sb.tile([128, 128], F32, tag="ew_zs", bufs=2, name=f"zs{l}_{t}")
                nc.vector.tensor_mul(out=zs[:], in0=zneg[:], in1=n[:])
                nc.vector.tensor_add(out=hown[l][:], in0=t1[:], in1=zs[:])
                hb = sb.tile([128, 128], BF16, tag="agc", bufs=2, name=f"agc{l}_{t}")
                nc.vector.tensor_copy(out=hb[:], in_=hown[l][:])
                tp = ps.tile([128, 4, 128], BF16, tag="tpc", bufs=2,
                             name=f"tph{l}_{t}")
                nc.tensor.transpose(tp[:, 0, :], hb[:], ident[:])
                hT = sb.tile([128, 128], BF16, tag="agT", bufs=2, name=f"agT{l}_{t}")
                nc.vector.tensor_copy(out=hT[:], in_=tp[:, 0, :])
                agin = dram.tile([128, B], BF16, tag=f"agin{l}", name=f"agin{l}_{t}")
                agout = dram.tile([n_cores * 128, B], BF16, tag=f"agout{l}",
                                  name=f"agout{l}_{t}")
                nc.sync.dma_start(agin[:], hT[:])
                nc.gpsimd.collective_compute(
                    "AllGather", ALU.bypass, replica_groups=rg,
                    ins=[agin.opt()], outs=[agout.opt()])
                ago = agout.rearrange("(ko ki) b -> ki ko b", ki=128)
                nc.sync.dma_start(hf[l][:, 0:4, :], ago[:, 0:4, :])
                nc.sync.dma_start(hf[l][:, 4:8, :], ago[:, 4:8, :])
                return hb

            # step-0 gate prologues (uses initial hidden)
            emit_gate_prologue(0, 0)
            emit_gate_prologue(1, 0)

            for t in range(steps):
                last = (t == steps - 1)
                # ---- phase A: x -> gates0 ----
                if t == 0:
                    emit_gih(0, t, [x0_sb[:, k, :] for k in range(KH)])
                else:
                    for g in range(2):
                        tp = ps.tile([128, 4, 128], BF16, tag="tpc", bufs=2,
                                     name=f"tpx_{t}_{g}")
                        for j in range(4):
                            k = 4 * g + j
                            nc.tensor.transpose(
                                tp[:, j, :], xgb[:, k * 128:(k + 1) * 128],
                                ident[:])
                        nc.vector.tensor_copy(out=xbf[:, 4 * g:4 * g + 4, :],
                                              in_=tp[:])
                    emit_gih(0, t, [xbf[:, k, :] for k in range(KH)])
                hT0 = emit_ew_ag(0, t)
                heat(heat_ag0, t, "ag0", hT0[:])

                # ---- phase B: h0 -> gates1 ----
                emit_gih(1, t, [hf[0][:, k, :] for k in range(KH)])
                hT1 = emit_ew_ag(1, t)
                heat(heat_ag1, t, "ag1", hT1[:])
                if not last:
                    # preload the Exp act table while AG1 flies so the first
                    # logits chunk's exp skips the 1.3us table switch
                    dmy = sb.tile([128, 2], F32, tag="dmy", bufs=1,
                                  name=f"dmyE_{t}")
                    nc.scalar.activation(dmy[:], ident[:, 0:2], AF.Exp)

                # ---- burst: logits (B,v), softmax over batch, xpart ----
                if not last:
                    xpA = ps.tile([128, 512], F32, tag="xpA", name=f"xpA_{t}")
                    xpB = ps.tile([128, 512], F32, tag="xpB", name=f"xpB_{t}")

                def emit_xpart(c):
                    for p in (2 * c, 2 * c + 1):
                        nc.tensor.matmul(xpA[:], expT8[:, 2 * p:2 * p + 2, :],
                                         win8[:, 2 * p:2 * p + 2, 0:512],
                                         start=(p == 0), stop=(p == 2 * NCH - 1),
                                         perf_mode=DR)
                        nc.tensor.matmul(xpB[:], expT8[:, 2 * p:2 * p + 2, :],
                                         win8[:, 2 * p:2 * p + 2, 512:1024],
                                         start=(p == 0), stop=(p == 2 * NCH - 1),
                                         perf_mode=DR)

                expcs = [None] * NCH

                def emit_tp_smgroup(c):
                    # PE-transpose the exp chunk to (v,B) psum, then sums,
                    # 1/s, and fp8 probs scale straight off psum
                    tpc = ps.tile([128, 4, 128], BF16, tag="tpc", bufs=2,
                                  name=f"tpe_{t}_{c}")
                    for j in range(4):
                        nc.tensor.transpose(
                            tpc[:, j, :], expcs[c][:, j * 128:(j + 1) * 128],
                            ident[:])
                    for j in range(4):
                        vo = 4 * c + j
                        nc.vector.tensor_reduce(
                            out=sums[:, vo:vo + 1], in_=tpc[:, j, :],
                            axis=mybir.AxisListType.X, op=ALU.add)
                    nc.vector.reciprocal(recs[:, 4 * c:4 * c + 4],
                                         sums[:, 4 * c:4 * c + 4])
                    for j in range(4):
                        vo = 4 * c + j
                        nc.vector.scalar_tensor_tensor(
                            out=expT8[:, vo, :], in0=tpc[:, j, :],
                            scalar=recs[:, vo:vo + 1], in1=c64[:],
                            op0=ALU.mult, op1=ALU.mult)

                for c in range(NCH):
                    lg = ps.tile([128, 512], F32, tag="lg", bufs=2, name=f"lg_{t}_{c}")
                    for k in range(KH):
                        nc.tensor.matmul(lg[:], hf[1][:, k, :],
                                         wout_sb[:, k, c * 512:(c + 1) * 512],
                                         start=(k == 0), stop=(k == KH - 1))
                    lout = sb.tile([128, 512], BF16, tag="lout", bufs=3,
                                   name=f"lout_{t}_{c}")
                    nc.vector.tensor_add(out=lout[:], in0=lg[:],
                                         in1=boutb[:, c * 512:(c + 1) * 512])
                    nc.sync.dma_start(d_out[t, :, c * 512:(c + 1) * 512], lout[:])
                    if not last:
                        expc = sb.tile([128, 512], BF16, tag="expc", bufs=3,
                                       name=f"expc_{t}_{c}")
                        nc.scalar.activation(expc[:], lout[:], AF.Exp)
                        expcs[c] = expc
                        if c >= 1:
                            emit_tp_smgroup(c - 1)
                        if c >= 3:
                            emit_xpart(c - 3)
                if not last:
                    # preload the Sigmoid table for the next step's ew
                    dmy = sb.tile([128, 2], F32, tag="dmy", bufs=1,
                                  name=f"dmyS_{t}")
                    nc.scalar.activation(dmy[:], ident[:, 0:2], AF.Sigmoid)
                    emit_tp_smgroup(NCH - 1)
                    for c in range(NCH - 3, NCH):
                        emit_xpart(c)
                    xstage = sb.tile([128, H], BF16, tag="xstage", bufs=2,
                                     name=f"xstage_{t}")
                    nc.vector.tensor_scalar_mul(xstage[:, 0:512], xpA[:], XDESC)
                    nc.vector.tensor_scalar_mul(xstage[:, 512:1024], xpB[:], XDESC)
                    arin = dram.tile([128, H], BF16, tag="arin", name=f"arin_{t}")
                    arout = dram.tile([128, H], BF16, tag="arout", name=f"arout_{t}")
                    nc.sync.dma_start(arin[:], xstage[:])
                    nc.gpsimd.collective_compute(
                        "AllReduce", ALU.add, replica_groups=rg,
                        ins=[arin.opt()], outs=[arout.opt()])
                    # next step's bias+ghh halves fill the AllReduce window
                    emit_gate_prologue(0, t + 1)
                    emit_gate_prologue(1, t + 1)
                    heat(heat_ar, t, "ar", xstage[:, 0:128])
                    nc.sync.dma_start(xgb[:, 0:512], arout[:, 0:512])
                    nc.sync.dma_start(xgb[:, 512:1024], arout[:, 512:1024])

    nc.compile()
    return nc


# ---------------- host side ----------------

def _prep_core_inputs(c, hidden, w_in, b_in, W_ih0, W_hh0, b_ih0, b_hh0,
                      W_ih1, W_hh1, b_ih1, b_hh1, w_out, b_out):
    bf = ml_dtypes.bfloat16
    f8 = ml_dtypes.float8_e4m3
    w_inT_pad = np.zeros((VPAD, H), np.float32)
    w_inT_pad[:V] = w_in.T
    w_outT_pad = np.zeros((H, VPAD), np.float32)
    w_outT_pad[:, :V] = w_out.T
    b_out_pad = np.zeros(VPAD, np.float32)
    b_out_pad[:V] = b_out

    d = {}
    wv = w_outT_pad[:, c * VS:(c + 1) * VS]              # (H, VS)
    d["woutT"] = np.ascontiguousarray(
        wv.reshape(KH, 128, VS).transpose(1, 0, 2)).astype(bf)
    d["boutb"] = np.ascontiguousarray(np.broadcast_to(
        b_out_pad[c * VS:(c + 1) * VS], (128, VS))).astype(bf)
    winv = w_inT_pad[c * VS:(c + 1) * VS, :] * WSC        # (VS, H)
    d["win8"] = np.ascontiguousarray(
        winv.reshape(VT, 128, H).transpose(1, 0, 2)).astype(f8)

    sel = np.concatenate([np.arange(c * 128, (c + 1) * 128) + g * H for g in range(3)])
    for nm, W in (("gih0", W_ih0), ("ghh0", W_hh0), ("gih1", W_ih1), ("ghh1", W_hh1)):
        Wsel = W[sel]                                     # (384, H)
        d[nm] = np.ascontiguousarray(
            Wsel.T.reshape(KH, 128, 384).transpose(1, 0, 2)).astype(bf)

    for l, (W_ih, b_ih, b_hh) in enumerate(((W_ih0, b_ih0, b_hh0),
                                            (W_ih1, b_ih1, b_hh1))):
        ih_eff = b_ih[sel].astype(np.float32)
        if l == 0:
            ih_eff = ih_eff + W_ih0[sel] @ b_in
        hh = b_hh[sel].astype(np.float32)
        row = np.zeros(512, np.float32)
        row[0:256] = ih_eff[0:256] + hh[0:256]            # r|z combined
        row[256:384] = ih_eff[256:384]                    # i_n bias
        row[384:512] = hh[256:384]                        # h_n bias
        d[f"brow{l}"] = row.reshape(1, 512)

    d["ones"] = np.ones((1, 128), np.float32)
    d["c64"] = np.full((128, 128), PSC, np.float32)

    x0 = w_inT_pad[BOS]                                   # (H,) == w_in[:, BOS]
    d["x0T"] = np.ascontiguousarray(
        np.broadcast_to(x0.reshape(KH, 128).T[:, :, None], (128, KH, B)))

    for l in range(2):
        hT = hidden[l].T                                  # (H, B)
        d[f"h{l}f"] = np.ascontiguousarray(
            hT.reshape(KH, 128, B).transpose(1, 0, 2)).astype(np.float32)
        d[f"h{l}own"] = np.ascontiguousarray(
            hidden[l][:, c * 128:(c + 1) * 128]).astype(np.float32)
    d["ident"] = np.eye(128, dtype=np.float32)

    casts = {"brow0": bf, "brow1": bf, "ones": bf, "c64": bf, "x0T": bf,
             "ident": bf}
    return {k: np.ascontiguousarray(v.astype(casts[k]) if k in casts else v)
            for k, v in d.items()}


_NC_CACHE = {}
BUILD_KW = {}


def _get_nc(steps=T):
    key = (steps, tuple(sorted(BUILD_KW.items())))
    if key not in _NC_CACHE:
        _NC_CACHE[key] = build_nc(steps, **BUILD_KW)
    return _NC_CACHE[key]


def kernel(**inputs):
    nc = _get_nc(T)
    in_maps = [_prep_core_inputs(c, **inputs) for c in range(NC)]
    res = bass_utils.run_bass_kernel_spmd(nc, in_maps, core_ids=list(range(NC)))
    out_pad = np.zeros((T, B, VPAD), np.float32)
    for c in range(NC):
        o = res.results[c]["logits"]                      # (T, 128, VS) bf16
        out_pad[:, :, c * VS:(c + 1) * VS] = np.asarray(o).astype(np.float32)
    return out_pad[:, :, :V]
